# revision 3
# baseline (speedup 1.0000x reference)
"""DCN cross-layer kernel for Trainium2 (8 NeuronCores, data-parallel).

Reference computes, for i = 0..L-1:
    x_{i+1} = x0 * (x_i . w_i) + b_i + x_i         (x0 fixed, per-row dot)

Algebraic collapse: every iterate has the form x_i = alpha_i * x0 + beta_i
with per-row scalar alpha_i and a row-independent vector beta_i:
    alpha_0 = 1,  beta_0 = 0
    alpha_{i+1} = alpha_i * (1 + c_i) + gamma_i,   c_i = x0 . w_i (per row)
    beta_{i+1}  = beta_i + b_i,                    gamma_i = beta_i . w_i
    out = alpha_L * x0 + beta_L

So the whole module reduces to one skinny matmul C = x0 @ W^T (B x L), a
tiny per-row recurrence over L=4 scalars, and one fused scale-add pass.
x is read once from HBM and the output written once — memory-roofline shape.

Per core (4096 rows): for each 128-row tile, PE transposes the tile
(8 x 128x128, via identity matmul) into PSUM, ACT copies it back to SBUF,
PE accumulates C = xT^T @ W^T chunks into PSUM, DVE runs the alpha
recurrence and the single fused out = (x0 * alpha) + beta pass.

Sharding: batch dim of x split across the 8 cores; the tiny (L,D)-derived
tensors (W^T chunks, beta_L, gammas) are replicated.
"""

import numpy as np

import concourse.bass as bass
import concourse.tile as tile
from concourse import mybir
from concourse.bass_utils import run_bass_kernel_spmd
from concourse.masks import make_identity
from concourse.vector_clock import ScopedClock

F32 = mybir.dt.float32
AL = mybir.AluOpType

B, D, L = 32768, 1024, 4
N_CORES = 8
BC = B // N_CORES          # rows per core
P = 128                    # SBUF partitions
NCHUNK = D // P            # 8 column chunks of 128
NT = BC // P               # 32 row-tiles per core


class SplitDrainTileContext(tile.TileContext):
    """The walrus build in this container rejects >4 sync waits on a single
    instruction, but the stock kernel-tail drain funnels every outstanding
    proc's wait onto one SP Drain. Redistribute them into a chain of
    single-wait drains (semantically identical: SP waits for each proc in
    turn before the exit barrier)."""

    MAXW = 1

    def _drain_and_barrier(self, tick_clock, wait_clock):
        drain_inst = self.nc.sync.drain()
        wait_clock.add_sem_waits(
            drain_inst.ins, ScopedClock({None: tick_clock.global_clock})
        )
        si = drain_inst.ins.sync_info
        waits = list(si.on_wait) if si is not None and si.on_wait else []
        if len(waits) > self.MAXW:
            drain_inst.ins.sync_info = mybir.SyncInfo(
                on_wait=waits[: self.MAXW],
                on_update=list(si.on_update or []),
            )
            rest = waits[self.MAXW:]
            for i in range(0, len(rest), self.MAXW):
                d2 = self.nc.sync.drain()
                d2.ins.sync_info = mybir.SyncInfo(
                    on_wait=rest[i : i + self.MAXW], on_update=[]
                )
        self.nc.all_engine_barrier()
        assert self.sems is not None
        popped = self.nc._tile_sem_poison_stack.pop()
        assert popped is self._sem_poison
        self.nc.clear_and_free_semaphores(list(self.sems.allocated().values()))
        self.nc.all_engine_barrier()


def _split_multiwait_insts(nc, maxw=1):
    """Walrus here rejects instructions carrying more than a few sync waits.
    Hoist excess waits onto single-wait NOPs inserted just before the
    offending instruction on the same engine (identical blocking
    semantics: the engine waits on each sem in turn)."""
    for bb in nc.main_func.blocks:
        insts = list(bb.bb.instructions if hasattr(bb, "bb") else bb.instructions)
        changed = False
        new = []
        for ins in insts:
            si = getattr(ins, "sync_info", None)
            waits = list(si.on_wait) if si is not None and si.on_wait else []
            if len(waits) > maxw and ins.engine != mybir.EngineType.Unassigned:
                extra, keep = waits[:-maxw], waits[-maxw:]
                for k in range(0, len(extra), maxw):
                    nop = mybir.InstNoOp(
                        name=nc.get_next_instruction_name(), ins=[], outs=[]
                    )
                    nop.engine = ins.engine
                    nop.sync_info = mybir.SyncInfo(
                        on_wait=extra[k : k + maxw], on_update=[]
                    )
                    new.append(nop)
                ins.sync_info = mybir.SyncInfo(
                    on_wait=keep, on_update=list(si.on_update or [])
                )
                changed = True
            new.append(ins)
        if changed:
            container = bb.bb if hasattr(bb, "bb") else bb
            container.instructions.clear()
            for ins in new:
                container.instructions.append(ins)


def build_kernel():
    nc = bass.Bass(target_bir_lowering=False)
    x_d = nc.dram_tensor("x", [BC, D], F32, kind="ExternalInput")
    # wt[p, j, l] = W[l, 128*j + p]  (host-pretransposed W^T, chunked)
    wt_d = nc.dram_tensor("wt", [P, NCHUNK, L], F32, kind="ExternalInput")
    beta_d = nc.dram_tensor("beta", [1, D], F32, kind="ExternalInput")
    gam_d = nc.dram_tensor("gam", [1, L], F32, kind="ExternalInput")
    out_d = nc.dram_tensor("out", [BC, D], F32, kind="ExternalOutput")

    with SplitDrainTileContext(nc) as tc:
        with (
            tc.tile_pool(name="consts", bufs=1) as consts,
            tc.tile_pool(name="xp", bufs=6) as xp,
            tc.tile_pool(name="xtp", bufs=3) as xtp,
            tc.tile_pool(name="op", bufs=4) as op,
            tc.tile_pool(name="small", bufs=4) as small,
            tc.tile_pool(name="pst", bufs=2, space="PSUM") as pst,
            tc.tile_pool(name="psc", bufs=2, space="PSUM") as psc,
        ):
            wt_sb = consts.tile([P, NCHUNK, L], F32)
            nc.sync.dma_start(wt_sb[:], wt_d[:, :, :])
            beta_sb = consts.tile([P, D], F32)
            nc.gpsimd.dma_start(beta_sb[:], beta_d[:, :].to_broadcast((P, D)))
            gam_sb = consts.tile([P, L], F32)
            nc.gpsimd.dma_start(gam_sb[:], gam_d[:, :].to_broadcast((P, L)))
            ident = consts.tile([P, P], F32)
            make_identity(nc, ident)

            for t in range(NT):
                x_sb = xp.tile([P, D], F32)
                nc.sync.dma_start(x_sb[:], x_d[t * P:(t + 1) * P, :])

                # x tile transposed, chunk by chunk, PE -> PSUM
                xt_ps = pst.tile([P, NCHUNK, P], F32)
                for j in range(NCHUNK):
                    nc.tensor.transpose(
                        xt_ps[:, j, :], x_sb[:, j * P:(j + 1) * P], ident
                    )
                xt_sb = xtp.tile([P, NCHUNK, P], F32)
                nc.scalar.copy(xt_sb[:], xt_ps[:])

                # C[r, l] = sum_d x[r, d] W[l, d], accumulated over chunks
                c_ps = psc.tile([P, L], F32)
                for j in range(NCHUNK):
                    nc.tensor.matmul(
                        c_ps[:],
                        xt_sb[:, j, :],
                        wt_sb[:, j, :],
                        start=(j == 0),
                        stop=(j == NCHUNK - 1),
                    )

                # T_i = 1 + c_i ; alpha_{i+1} = alpha_i * T_i + gamma_i
                t_sb = small.tile([P, L], F32)
                nc.vector.tensor_scalar_add(out=t_sb[:], in0=c_ps[:], scalar1=1.0)
                al_sb = small.tile([P, L], F32)
                prev = t_sb[:, 0:1]  # alpha_1 (gamma_0 = 0)
                for i in range(1, L):
                    nc.vector.tensor_scalar(
                        out=al_sb[:, i:i + 1],
                        in0=t_sb[:, i:i + 1],
                        scalar1=prev,
                        scalar2=gam_sb[:, i:i + 1],
                        op0=AL.mult,
                        op1=AL.add,
                    )
                    prev = al_sb[:, i:i + 1]

                # out = alpha_L * x0 + beta_L, one fused DVE pass
                o_sb = op.tile([P, D], F32)
                nc.vector.scalar_tensor_tensor(
                    out=o_sb[:],
                    in0=x_sb[:],
                    scalar=prev,
                    in1=beta_sb[:],
                    op0=AL.mult,
                    op1=AL.add,
                )
                nc.sync.dma_start(out_d[t * P:(t + 1) * P, :], o_sb[:])
    _split_multiwait_insts(nc)
    return nc


_NC_CACHE = []


def _get_nc():
    if not _NC_CACHE:
        _NC_CACHE.append(build_kernel())
    return _NC_CACHE[0]


def prep_inputs(x, weights, biases):
    """Shard x by batch across cores; derive the tiny replicated tensors."""
    x = np.ascontiguousarray(np.asarray(x, dtype=np.float32))
    w = np.asarray(weights, dtype=np.float64)
    b = np.asarray(biases, dtype=np.float64)
    assert x.shape == (B, D) and w.shape == (L, D) and b.shape == (L, D)

    betas = np.concatenate([np.zeros((1, D)), np.cumsum(b, axis=0)], axis=0)
    gammas = np.array([betas[i] @ w[i] for i in range(L)])  # gamma_0 = 0
    beta_l = betas[L].astype(np.float32)[None, :]
    gam = gammas.astype(np.float32)[None, :]
    # wt[p, j, l] = W[l, 128*j + p]
    wt = np.ascontiguousarray(
        w.astype(np.float32).T.reshape(NCHUNK, P, L).transpose(1, 0, 2)
    )
    in_maps = [
        {"x": x[c * BC:(c + 1) * BC], "wt": wt, "beta": beta_l, "gam": gam}
        for c in range(N_CORES)
    ]
    return in_maps


def run_sharded(x, weights, biases, **run_kwargs):
    nc = _get_nc()
    in_maps = prep_inputs(x, weights, biases)
    res = run_bass_kernel_spmd(nc, in_maps, core_ids=list(range(N_CORES)), **run_kwargs)
    out = np.concatenate([r["out"] for r in res.results], axis=0)
    return out, res


def kernel(x, weights, biases):
    out, _ = run_sharded(x, weights, biases)
    return out


# revision 7
# speedup vs baseline: 37910.3631x; 37910.3631x over previous
"""DCN cross-layer kernel for Trainium2 (8 NeuronCores, data-parallel).

Reference computes, for i = 0..L-1:
    x_{i+1} = x0 * (x_i . w_i) + b_i + x_i         (x0 fixed, per-row dot)

Algebraic collapse: every iterate has the form x_i = alpha_i * x0 + beta_i
with per-row scalar alpha_i and a row-independent vector beta_i:
    alpha_0 = 1,  beta_0 = 0
    alpha_{i+1} = alpha_i * (1 + c_i) + gamma_i,   c_i = x0 . w_i (per row)
    beta_{i+1}  = beta_i + b_i,                    gamma_i = beta_i . w_i
    out = alpha_L * x0 + beta_L

So the whole module reduces to one skinny matmul C = x0 @ W^T (B x L), a
tiny per-row recurrence over L=4 scalars, and one fused scale-add pass.
x is read once from HBM and the output written once — memory-roofline shape.

Per core (4096 rows): for each 128-row tile, PE transposes the tile
(8 x 128x128, via identity matmul) into PSUM, ACT copies it back to SBUF,
PE accumulates C = xT^T @ W^T chunks into PSUM, DVE runs the alpha
recurrence and the single fused out = (x0 * alpha) + beta pass.

Sharding: batch dim of x split across the 8 cores; the tiny (L,D)-derived
tensors (W^T chunks, beta_L, gammas) are replicated.
"""

import numpy as np

import concourse.bass as bass
import concourse.tile as tile
from concourse import mybir
from concourse.bass_utils import run_bass_kernel_spmd
from concourse.masks import make_identity
from concourse.vector_clock import ScopedClock

F32 = mybir.dt.float32
AL = mybir.AluOpType

B, D, L = 32768, 1024, 4
N_CORES = 8
BC = B // N_CORES          # rows per core
P = 128                    # SBUF partitions
NCHUNK = D // P            # 8 column chunks of 128
NT = BC // P               # 32 row-tiles per core


class SplitDrainTileContext(tile.TileContext):
    """The walrus build in this container rejects >4 sync waits on a single
    instruction, but the stock kernel-tail drain funnels every outstanding
    proc's wait onto one SP Drain. Redistribute them into a chain of
    single-wait drains (semantically identical: SP waits for each proc in
    turn before the exit barrier)."""

    MAXW = 1

    def _drain_and_barrier(self, tick_clock, wait_clock):
        drain_inst = self.nc.sync.drain()
        wait_clock.add_sem_waits(
            drain_inst.ins, ScopedClock({None: tick_clock.global_clock})
        )
        si = drain_inst.ins.sync_info
        waits = list(si.on_wait) if si is not None and si.on_wait else []
        if len(waits) > self.MAXW:
            drain_inst.ins.sync_info = mybir.SyncInfo(
                on_wait=waits[: self.MAXW],
                on_update=list(si.on_update or []),
            )
            rest = waits[self.MAXW:]
            for i in range(0, len(rest), self.MAXW):
                d2 = self.nc.sync.drain()
                d2.ins.sync_info = mybir.SyncInfo(
                    on_wait=rest[i : i + self.MAXW], on_update=[]
                )
        self.nc.all_engine_barrier()
        assert self.sems is not None
        popped = self.nc._tile_sem_poison_stack.pop()
        assert popped is self._sem_poison
        self.nc.clear_and_free_semaphores(list(self.sems.allocated().values()))
        self.nc.all_engine_barrier()


def _split_multiwait_insts(nc, maxw=1):
    """Walrus here rejects instructions carrying more than a few sync waits.
    Hoist excess waits onto single-wait NOPs inserted just before the
    offending instruction on the same engine (identical blocking
    semantics: the engine waits on each sem in turn)."""
    for bb in nc.main_func.blocks:
        insts = list(bb.bb.instructions if hasattr(bb, "bb") else bb.instructions)
        changed = False
        new = []
        for ins in insts:
            si = getattr(ins, "sync_info", None)
            waits = list(si.on_wait) if si is not None and si.on_wait else []
            if len(waits) > maxw and ins.engine != mybir.EngineType.Unassigned:
                extra, keep = waits[:-maxw], waits[-maxw:]
                for k in range(0, len(extra), maxw):
                    nop = mybir.InstNoOp(
                        name=nc.get_next_instruction_name(), ins=[], outs=[]
                    )
                    nop.engine = ins.engine
                    nop.sync_info = mybir.SyncInfo(
                        on_wait=extra[k : k + maxw], on_update=[]
                    )
                    new.append(nop)
                ins.sync_info = mybir.SyncInfo(
                    on_wait=keep, on_update=list(si.on_update or [])
                )
                changed = True
            new.append(ins)
        if changed:
            container = bb.bb if hasattr(bb, "bb") else bb
            container.instructions.clear()
            for ins in new:
                container.instructions.append(ins)


def build_kernel(repeat=1):
    """repeat>1 wraps the whole tile loop in a dynamic For_i that re-runs it
    (same inputs/outputs) -- used only for on-device timing, where wall-clock
    differencing of two repeat counts cancels dispatch/transfer overhead."""
    nc = bass.Bass(target_bir_lowering=False)
    x_d = nc.dram_tensor("x", [BC, D], F32, kind="ExternalInput")
    # wt[p, j, l] = W[l, 128*j + p]  (host-pretransposed W^T, chunked)
    wt_d = nc.dram_tensor("wt", [P, NCHUNK, L], F32, kind="ExternalInput")
    beta_d = nc.dram_tensor("beta", [1, D], F32, kind="ExternalInput")
    gam_d = nc.dram_tensor("gam", [1, L], F32, kind="ExternalInput")
    out_d = nc.dram_tensor("out", [BC, D], F32, kind="ExternalOutput")

    with SplitDrainTileContext(nc) as tc:
        with (
            tc.tile_pool(name="consts", bufs=1) as consts,
            tc.tile_pool(name="xp", bufs=6) as xp,
            tc.tile_pool(name="xtp", bufs=3) as xtp,
            tc.tile_pool(name="op", bufs=4) as op,
            tc.tile_pool(name="small", bufs=4) as small,
            tc.tile_pool(name="pst", bufs=2, space="PSUM") as pst,
            tc.tile_pool(name="psc", bufs=2, space="PSUM") as psc,
        ):
            wt_sb = consts.tile([P, NCHUNK, L], F32)
            nc.sync.dma_start(wt_sb[:], wt_d[:, :, :])
            beta_sb = consts.tile([P, D], F32)
            nc.gpsimd.dma_start(beta_sb[:], beta_d[:, :].to_broadcast((P, D)))
            gam_sb = consts.tile([P, L], F32)
            nc.gpsimd.dma_start(gam_sb[:], gam_d[:, :].to_broadcast((P, L)))
            ident = consts.tile([P, P], F32)
            make_identity(nc, ident)

            import contextlib

            rep_ctx = (
                tc.For_i(0, repeat, 1) if repeat > 1 else contextlib.nullcontext()
            )
            with rep_ctx:
                _tile_loop(nc, tc, x_d, out_d, wt_sb, beta_sb, gam_sb, ident,
                           xp, xtp, op, small, pst, psc)
    _split_multiwait_insts(nc)
    return nc


def _tile_loop(nc, tc, x_d, out_d, wt_sb, beta_sb, gam_sb, ident,
               xp, xtp, op, small, pst, psc):
            for t in range(NT):
                x_sb = xp.tile([P, D], F32)
                nc.sync.dma_start(x_sb[:], x_d[t * P:(t + 1) * P, :])

                # x tile transposed, chunk by chunk, PE -> PSUM
                xt_ps = pst.tile([P, NCHUNK, P], F32)
                for j in range(NCHUNK):
                    nc.tensor.transpose(
                        xt_ps[:, j, :], x_sb[:, j * P:(j + 1) * P], ident
                    )
                xt_sb = xtp.tile([P, NCHUNK, P], F32)
                nc.scalar.copy(xt_sb[:], xt_ps[:])

                # C[r, l] = sum_d x[r, d] W[l, d], accumulated over chunks
                c_ps = psc.tile([P, L], F32)
                for j in range(NCHUNK):
                    nc.tensor.matmul(
                        c_ps[:],
                        xt_sb[:, j, :],
                        wt_sb[:, j, :],
                        start=(j == 0),
                        stop=(j == NCHUNK - 1),
                    )

                # T_i = 1 + c_i ; alpha_{i+1} = alpha_i * T_i + gamma_i
                t_sb = small.tile([P, L], F32)
                nc.vector.tensor_scalar_add(out=t_sb[:], in0=c_ps[:], scalar1=1.0)
                al_sb = small.tile([P, L], F32)
                prev = t_sb[:, 0:1]  # alpha_1 (gamma_0 = 0)
                for i in range(1, L):
                    nc.vector.tensor_scalar(
                        out=al_sb[:, i:i + 1],
                        in0=t_sb[:, i:i + 1],
                        scalar1=prev,
                        scalar2=gam_sb[:, i:i + 1],
                        op0=AL.mult,
                        op1=AL.add,
                    )
                    prev = al_sb[:, i:i + 1]

                # out = alpha_L * x0 + beta_L, one fused DVE pass
                o_sb = op.tile([P, D], F32)
                nc.vector.scalar_tensor_tensor(
                    out=o_sb[:],
                    in0=x_sb[:],
                    scalar=prev,
                    in1=beta_sb[:],
                    op0=AL.mult,
                    op1=AL.add,
                )
                nc.sync.dma_start(out_d[t * P:(t + 1) * P, :], o_sb[:])


_NC_CACHE = []


def _get_nc():
    if not _NC_CACHE:
        _NC_CACHE.append(build_kernel())
    return _NC_CACHE[0]


def prep_inputs(x, weights, biases):
    """Shard x by batch across cores; derive the tiny replicated tensors."""
    x = np.ascontiguousarray(np.asarray(x, dtype=np.float32))
    w = np.asarray(weights, dtype=np.float64)
    b = np.asarray(biases, dtype=np.float64)
    assert x.shape == (B, D) and w.shape == (L, D) and b.shape == (L, D)

    betas = np.concatenate([np.zeros((1, D)), np.cumsum(b, axis=0)], axis=0)
    gammas = np.array([betas[i] @ w[i] for i in range(L)])  # gamma_0 = 0
    beta_l = betas[L].astype(np.float32)[None, :]
    gam = gammas.astype(np.float32)[None, :]
    # wt[p, j, l] = W[l, 128*j + p]
    wt = np.ascontiguousarray(
        w.astype(np.float32).T.reshape(NCHUNK, P, L).transpose(1, 0, 2)
    )
    in_maps = [
        {"x": x[c * BC:(c + 1) * BC], "wt": wt, "beta": beta_l, "gam": gam}
        for c in range(N_CORES)
    ]
    return in_maps


def run_sharded(x, weights, biases, **run_kwargs):
    nc = _get_nc()
    in_maps = prep_inputs(x, weights, biases)
    res = run_bass_kernel_spmd(nc, in_maps, core_ids=list(range(N_CORES)), **run_kwargs)
    out = np.concatenate([r["out"] for r in res.results], axis=0)
    return out, res


def kernel(x, weights, biases):
    out, _ = run_sharded(x, weights, biases)
    return out


# revision 17
# speedup vs baseline: 62609.5962x; 1.6515x over previous
"""DCN cross-layer kernel for Trainium2 (8 NeuronCores, data-parallel).

Reference computes, for i = 0..L-1:
    x_{i+1} = x0 * (x_i . w_i) + b_i + x_i         (x0 fixed, per-row dot)

Algebraic collapse: every iterate has the form x_i = alpha_i * x0 + beta_i
with per-row scalar alpha_i and a row-independent vector beta_i:
    alpha_0 = 1,  beta_0 = 0
    alpha_{i+1} = alpha_i * (1 + c_i) + gamma_i,   c_i = x0 . w_i (per row)
    beta_{i+1}  = beta_i + b_i,                    gamma_i = beta_i . w_i
    out = alpha_L * x0 + beta_L

So the whole module reduces to one skinny matmul C = x0 @ W^T (B x L), a
tiny per-row recurrence over L=4 scalars, and one fused scale-add pass.
x is read once from HBM and the output written once — memory-roofline shape.

Per core (4096 rows): for each 128-row tile, PE transposes the tile
(8 x 128x128, via identity matmul) into PSUM, ACT copies it back to SBUF,
PE accumulates C = xT^T @ W^T chunks into PSUM, DVE runs the alpha
recurrence and the single fused out = (x0 * alpha) + beta pass.

Sharding: batch dim of x split across the 8 cores; the tiny (L,D)-derived
tensors (W^T chunks, beta_L, gammas) are replicated.
"""

import numpy as np

import concourse.bass as bass
import concourse.tile as tile
from concourse import mybir
from concourse.bass_utils import run_bass_kernel_spmd
from concourse.masks import make_identity
from concourse.vector_clock import ScopedClock

F32 = mybir.dt.float32
AL = mybir.AluOpType

B, D, L = 32768, 1024, 4
N_CORES = 8
BC = B // N_CORES          # rows per core
P = 128                    # SBUF partitions
NCHUNK = D // P            # 8 column chunks of 128
NT = BC // P               # 32 row-tiles per core

# Engine split for the C = x @ W^T dot products: PE handles d-chunks
# [0, K_PE*128) via transpose+matmul; DVE handles the tail d-range via
# fused multiply-reduce (tensor_tensor_reduce). Balances PE vs DVE so both
# hide under the ~95us/core HBM floor.
K_PE = NCHUNK - 2          # 6 chunks on PE
D_PE = K_PE * P            # 768
D_DVE = D - D_PE           # 256


class SplitDrainTileContext(tile.TileContext):
    """The walrus build in this container rejects >4 sync waits on a single
    instruction, but the stock kernel-tail drain funnels every outstanding
    proc's wait onto one SP Drain. Redistribute them into a chain of
    single-wait drains (semantically identical: SP waits for each proc in
    turn before the exit barrier)."""

    MAXW = 1

    def _drain_and_barrier(self, tick_clock, wait_clock):
        drain_inst = self.nc.sync.drain()
        wait_clock.add_sem_waits(
            drain_inst.ins, ScopedClock({None: tick_clock.global_clock})
        )
        si = drain_inst.ins.sync_info
        waits = list(si.on_wait) if si is not None and si.on_wait else []
        if len(waits) > self.MAXW:
            drain_inst.ins.sync_info = mybir.SyncInfo(
                on_wait=waits[: self.MAXW],
                on_update=list(si.on_update or []),
            )
            rest = waits[self.MAXW:]
            for i in range(0, len(rest), self.MAXW):
                d2 = self.nc.sync.drain()
                d2.ins.sync_info = mybir.SyncInfo(
                    on_wait=rest[i : i + self.MAXW], on_update=[]
                )
        self.nc.all_engine_barrier()
        assert self.sems is not None
        popped = self.nc._tile_sem_poison_stack.pop()
        assert popped is self._sem_poison
        self.nc.clear_and_free_semaphores(list(self.sems.allocated().values()))
        self.nc.all_engine_barrier()


def _split_multiwait_insts(nc, maxw=1):
    """Walrus here rejects instructions carrying more than a few sync waits.
    Hoist excess waits onto single-wait NOPs inserted just before the
    offending instruction on the same engine (identical blocking
    semantics: the engine waits on each sem in turn)."""
    for bb in nc.main_func.blocks:
        insts = list(bb.bb.instructions if hasattr(bb, "bb") else bb.instructions)
        changed = False
        new = []
        for ins in insts:
            si = getattr(ins, "sync_info", None)
            waits = list(si.on_wait) if si is not None and si.on_wait else []
            if len(waits) > maxw and ins.engine != mybir.EngineType.Unassigned:
                extra, keep = waits[:-maxw], waits[-maxw:]
                for k in range(0, len(extra), maxw):
                    nop = mybir.InstNoOp(
                        name=nc.get_next_instruction_name(), ins=[], outs=[]
                    )
                    nop.engine = ins.engine
                    nop.sync_info = mybir.SyncInfo(
                        on_wait=extra[k : k + maxw], on_update=[]
                    )
                    new.append(nop)
                ins.sync_info = mybir.SyncInfo(
                    on_wait=keep, on_update=list(si.on_update or [])
                )
                changed = True
            new.append(ins)
        if changed:
            container = bb.bb if hasattr(bb, "bb") else bb
            container.instructions.clear()
            for ins in new:
                container.instructions.append(ins)


def build_kernel(repeat=1):
    """repeat>1 wraps the whole tile loop in a dynamic For_i that re-runs it
    (same inputs/outputs) -- used only for on-device timing, where wall-clock
    differencing of two repeat counts cancels dispatch/transfer overhead."""
    nc = bass.Bass(target_bir_lowering=False)
    x_d = nc.dram_tensor("x", [BC, D], F32, kind="ExternalInput")
    # wt[p, j, l] = W[l, 128*j + p]  (host-pretransposed W^T, chunked)
    wt_d = nc.dram_tensor("wt", [P, K_PE, L], F32, kind="ExternalInput")
    # wb[0, l*D_DVE + m] = W[l, D_PE + m]  (tail chunks, row-broadcast)
    wb_d = nc.dram_tensor("wb", [1, L * D_DVE], F32, kind="ExternalInput")
    beta_d = nc.dram_tensor("beta", [1, D], F32, kind="ExternalInput")
    gam_d = nc.dram_tensor("gam", [1, L], F32, kind="ExternalInput")
    out_d = nc.dram_tensor("out", [BC, D], F32, kind="ExternalOutput")

    with SplitDrainTileContext(nc) as tc:
        with (
            tc.tile_pool(name="consts", bufs=1) as consts,
            tc.tile_pool(name="xp", bufs=6) as xp,
            tc.tile_pool(name="xtp", bufs=3) as xtp,
            tc.tile_pool(name="op", bufs=4) as op,
            tc.tile_pool(name="small", bufs=4) as small,
            tc.tile_pool(name="pst", bufs=2, space="PSUM") as pst,
            tc.tile_pool(name="psc", bufs=2, space="PSUM") as psc,
        ):
            wt_sb = consts.tile([P, K_PE, L], F32)
            nc.sync.dma_start(wt_sb[:], wt_d[:, :, :])
            wb_sb = consts.tile([P, L * D_DVE], F32)
            nc.gpsimd.dma_start(
                wb_sb[:], wb_d[:, :].to_broadcast((P, L * D_DVE))
            )
            beta_sb = consts.tile([P, D], F32)
            nc.gpsimd.dma_start(beta_sb[:], beta_d[:, :].to_broadcast((P, D)))
            gam_sb = consts.tile([P, L], F32)
            nc.gpsimd.dma_start(gam_sb[:], gam_d[:, :].to_broadcast((P, L)))
            ident = consts.tile([P, P], F32)
            make_identity(nc, ident)

            import contextlib

            rep_ctx = (
                tc.For_i(0, repeat, 1) if repeat > 1 else contextlib.nullcontext()
            )
            with rep_ctx:
                _tile_loop(nc, tc, x_d, out_d, wt_sb, wb_sb, beta_sb, gam_sb,
                           ident, xp, xtp, op, small, pst, psc)
    _split_multiwait_insts(nc)
    return nc


def _tile_loop(nc, tc, x_d, out_d, wt_sb, wb_sb, beta_sb, gam_sb, ident,
               xp, xtp, op, small, pst, psc):
    for t in range(NT):
        x_sb = xp.tile([P, D], F32)
        nc.sync.dma_start(x_sb[:], x_d[t * P:(t + 1) * P, :])

        # head chunks transposed on PE -> PSUM, ACT copies back to SBUF
        xt_ps = pst.tile([P, K_PE, P], F32)
        for j in range(K_PE):
            nc.tensor.transpose(
                xt_ps[:, j, :], x_sb[:, j * P:(j + 1) * P], ident
            )
        xt_sb = xtp.tile([P, K_PE, P], F32)
        nc.scalar.copy(xt_sb[:], xt_ps[:])

        # PE partial dot: c_pe[r, l] = sum_{d < D_PE} x[r, d] W[l, d]
        c_ps = psc.tile([P, L], F32)
        for j in range(K_PE):
            nc.tensor.matmul(
                c_ps[:],
                xt_sb[:, j, :],
                wt_sb[:, j, :],
                start=(j == 0),
                stop=(j == K_PE - 1),
            )
        cp_sb = small.tile([P, L], F32)
        nc.scalar.copy(out=cp_sb[:], in_=c_ps[:])

        # DVE finishes the dot over the tail d-range: fused multiply with
        # free-axis sum into ct_sb (one pass per layer)
        ct_sb = small.tile([P, L], F32)
        prod = xtp.tile([P, D_DVE], F32, tag="prod")
        for l in range(L):
            nc.vector.scalar_tensor_tensor(
                out=prod[:],
                in0=x_sb[:, D_PE:],
                scalar=1.0,
                in1=wb_sb[:, l * D_DVE:(l + 1) * D_DVE],
                op0=AL.mult,
                op1=AL.mult,
                accum_out=ct_sb[:, l:l + 1],
            )

        # T_i = 1 + c_i with c_i = c_pe + c_tail, fused in one tiny op
        t_sb = small.tile([P, L], F32)
        nc.vector.scalar_tensor_tensor(
            out=t_sb[:],
            in0=cp_sb[:],
            scalar=1.0,
            in1=ct_sb[:],
            op0=AL.add,
            op1=AL.add,
        )
        al_sb = small.tile([P, L], F32)
        prev = t_sb[:, 0:1]  # alpha_1 (gamma_0 = 0)
        for i in range(1, L):
            nc.vector.tensor_scalar(
                out=al_sb[:, i:i + 1],
                in0=t_sb[:, i:i + 1],
                scalar1=prev,
                scalar2=gam_sb[:, i:i + 1],
                op0=AL.mult,
                op1=AL.add,
            )
            prev = al_sb[:, i:i + 1]

        # out = alpha_L * x0 + beta_L, one fused DVE pass
        o_sb = op.tile([P, D], F32)
        nc.vector.scalar_tensor_tensor(
            out=o_sb[:],
            in0=x_sb[:],
            scalar=prev,
            in1=beta_sb[:],
            op0=AL.mult,
            op1=AL.add,
        )
        nc.sync.dma_start(out_d[t * P:(t + 1) * P, :], o_sb[:])


_NC_CACHE = []


def _get_nc():
    if not _NC_CACHE:
        _NC_CACHE.append(build_kernel())
    return _NC_CACHE[0]


def prep_inputs(x, weights, biases):
    """Shard x by batch across cores; derive the tiny replicated tensors."""
    x = np.ascontiguousarray(np.asarray(x, dtype=np.float32))
    w = np.asarray(weights, dtype=np.float64)
    b = np.asarray(biases, dtype=np.float64)
    assert x.shape == (B, D) and w.shape == (L, D) and b.shape == (L, D)

    betas = np.concatenate([np.zeros((1, D)), np.cumsum(b, axis=0)], axis=0)
    gammas = np.array([betas[i] @ w[i] for i in range(L)])  # gamma_0 = 0
    beta_l = betas[L].astype(np.float32)[None, :]
    gam = gammas.astype(np.float32)[None, :]
    wf = w.astype(np.float32)
    # wt[p, j, l] = W[l, 128*j + p] for the PE chunks
    wt = np.ascontiguousarray(
        wf[:, :D_PE].T.reshape(K_PE, P, L).transpose(1, 0, 2)
    )
    # wb[0, l*D_DVE + m] = W[l, D_PE + m] for the DVE tail
    wb = np.ascontiguousarray(wf[:, D_PE:].reshape(1, L * D_DVE))
    in_maps = [
        {
            "x": x[c * BC:(c + 1) * BC],
            "wt": wt,
            "wb": wb,
            "beta": beta_l,
            "gam": gam,
        }
        for c in range(N_CORES)
    ]
    return in_maps


def run_sharded(x, weights, biases, **run_kwargs):
    nc = _get_nc()
    in_maps = prep_inputs(x, weights, biases)
    res = run_bass_kernel_spmd(nc, in_maps, core_ids=list(range(N_CORES)), **run_kwargs)
    out = np.concatenate([r["out"] for r in res.results], axis=0)
    return out, res


def kernel(x, weights, biases):
    out, _ = run_sharded(x, weights, biases)
    return out
